# revision 16
# baseline (speedup 1.0000x reference)
"""AdaptiveJacobianPrunedViT — Trainium2 Bass kernel (8 NeuronCores).

Strategy
--------
Data-parallel over batch: B=8 images, one per core. Each core runs the full
12-layer ViT on its image with true token compaction (pruned tokens are
physically gathered out via indirect DMA between layers), so later layers
shrink per the pruning schedule. The schedule and keep-index lists are the
model's data-dependent control flow; the reference itself resolves them with
CPU syncs, and we do the same: a cheap fp32 numpy replica on the host derives
(T_l, keep_idx_l), which enter the device graph as shapes and int32 gather
inputs. keep_idx is shared across the batch (batch-mean importance), so all
cores gather identically and shards never diverge.

Device numerics: fp16 GEMM operands with fp32 PSUM accumulation; the residual
stream, LN statistics and softmax normalization stay fp32. LN scale/bias are
folded into adjacent weights on the host. Per-query softmax sums ride on the
AV matmul via a ones-column appended to V.
"""

import sys
import types
import numpy as np

import concourse.bass as bass
import concourse.mybir as mybir
import concourse.tile as tile
from concourse import bacc
from concourse.bass_utils import run_bass_kernel_spmd
from concourse.masks import make_identity
from concourse.vector_clock import ScopedClock, VectorClock

F16 = mybir.dt.float16
F32 = mybir.dt.float32
I32 = mybir.dt.int32
AF = mybir.ActivationFunctionType

B, C, IMG, P = 8, 3, 384, 16
D, H, L, MLP, NCLS = 384, 6, 12, 1536, 1000
G = IMG // P
T0 = G * G + 1  # 577
HD = D // H  # 64
GAMMA, MIN_TOKENS, EPS = 0.1, 16, 1e-6
LN_EPS = 1e-5
SCALE = HD ** -0.5

# Expected per-layer token counts for the canonical seed-0 inputs (recomputed
# at runtime by the host pre-pass; listed for reference/cache warmth).
EXPECTED_SCHED = [577, 577, 519, 467, 420, 377, 339, 305, 274, 246, 221, 198]


def _pad128(n):
    return (n + 127) // 128 * 128


# --------------------------------------------------------------------------
# Tile tail-drain patch: this walrus encodes at most one sync wait on a CTRL
# instruction; TileContext's kernel-tail drain attaches one wait per active
# logical proc. Split them across sync-engine nops (program order on SP
# preserves the barrier semantics).
# --------------------------------------------------------------------------
def _patched_drain_and_barrier(self, tick_clock, wait_clock):
    gc = tick_clock.global_clock
    for p, t in enumerate(list(gc)):
        if t > 0:
            nop = self.nc.sync.nop()
            vc = VectorClock()
            vc.require_at_least(p, t)
            wait_clock.add_sem_waits(nop.ins, ScopedClock({None: vc}))
    self.nc.sync.drain()
    self.nc.all_engine_barrier()
    popped = self.nc._tile_sem_poison_stack.pop()
    assert popped is self._sem_poison
    self.nc.clear_and_free_semaphores(list(self.sems.allocated().values()))
    self.nc.all_engine_barrier()


def _install_patches():
    tile.TileContext._drain_and_barrier = _patched_drain_and_barrier


# --------------------------------------------------------------------------
# Host pre-pass: fp32 numpy replica of the reference, used ONLY to derive the
# pruning schedule + keep-index lists. The device computes the output.
# --------------------------------------------------------------------------
def _gelu(x):
    try:
        from scipy.special import erf
        return (0.5 * x * (1.0 + erf(x / np.float32(np.sqrt(2.0))))).astype(x.dtype)
    except ImportError:  # pragma: no cover
        import math
        v = np.vectorize(math.erf, otypes=[np.float32])
        return (0.5 * x * (1.0 + v(x / np.float32(np.sqrt(2.0))))).astype(np.float32)


def _ln_np(x, s, b):
    m = x.mean(-1, keepdims=True)
    v = ((x - m) ** 2).mean(-1, keepdims=True)
    return (x - m) / np.sqrt(v + LN_EPS) * s + b


def _softmax_np(x):
    x = x - x.max(-1, keepdims=True)
    e = np.exp(x)
    return e / e.sum(-1, keepdims=True)


def _host_schedule(inputs):
    """Returns (T_per_layer, keeps): keeps[l] is the sorted keep index array
    (into layer-l tokens, CLS included) applied AFTER layer l, or None."""
    x = np.asarray(inputs['x'], np.float32)
    Bc = x.shape[0]
    patches = x.reshape(Bc, C, G, P, G, P).transpose(0, 2, 4, 1, 3, 5).reshape(Bc, G * G, C * P * P)
    tok = patches @ inputs['patch_w'] + inputs['patch_b']
    xcur = np.concatenate(
        [np.broadcast_to(np.asarray(inputs['cls_token'], np.float32), (Bc, 1, D)), tok], axis=1
    ) + inputs['pos_embed']
    N = xcur.shape[1] - 1
    prev_mass = None
    sched_T = []
    keeps = []
    for l in range(L):
        Tt = xcur.shape[1]
        sched_T.append(Tt)
        xn = _ln_np(xcur, inputs['ln1_s'][l], inputs['ln1_b'][l])
        qkv = (xn @ inputs['qkv_w'][l] + inputs['qkv_b'][l]).reshape(Bc, Tt, 3, H, HD).transpose(2, 0, 3, 1, 4)
        q, k, v = qkv[0], qkv[1], qkv[2]
        scores = np.einsum('bhqd,bhkd->bhqk', q, k) * np.float32(SCALE)
        attn = _softmax_np(scores)
        out = np.einsum('bhqk,bhkd->bhqd', attn, v).transpose(0, 2, 1, 3).reshape(Bc, Tt, D)
        xcur = xcur + out @ inputs['proj_w'][l] + inputs['proj_b'][l]
        xn2 = _ln_np(xcur, inputs['ln2_s'][l], inputs['ln2_b'][l])
        xcur = xcur + _gelu(xn2 @ inputs['fc1_w'][l] + inputs['fc1_b'][l]) @ inputs['fc2_w'][l] + inputs['fc2_b'][l]
        keep = None
        if N > MIN_TOKENS:
            cls = attn[:, :, 0, :]
            ent = -(cls * np.log(cls + EPS)).sum(-1)
            rho = (ent / np.log(np.float32(attn.shape[-1]))).mean(1)
            vnorm = np.linalg.norm(v, axis=-1)
            raw = (attn[:, :, 0, 1:] * vnorm[:, :, 1:]).sum(1)
            mass = raw.sum(-1)
            importance = raw / (mass[:, None] + EPS)
            if prev_mass is not None:
                delta = np.abs(mass - prev_mass) / (prev_mass + EPS)
                kr = float(np.clip(1.0 - GAMMA * (rho.mean() + delta.mean()), 0.0, 1.0))
                N_next = max(MIN_TOKENS, int(N * kr))
            else:
                N_next = N
            if N_next < N:
                s = importance.mean(0)
                order = np.argsort(-s, kind='stable')
                idx = order[:N_next]
                keep = np.concatenate([np.zeros((1,), np.int64), np.sort(idx) + 1]).astype(np.int32)
                xcur = xcur[:, keep]
                N = N_next
            prev_mass = mass
        keeps.append(keep)
    return sched_T, keeps


# --------------------------------------------------------------------------
# Host weight prep: fold LN scale/bias into adjacent GEMMs, cast to fp16.
# --------------------------------------------------------------------------
def _prep_weights(inputs):
    f32 = lambda a: np.asarray(a, np.float32)
    qkv_w, qkv_b = f32(inputs['qkv_w']), f32(inputs['qkv_b'])
    proj_w, proj_b = f32(inputs['proj_w']), f32(inputs['proj_b'])
    fc1_w, fc1_b = f32(inputs['fc1_w']), f32(inputs['fc1_b'])
    fc2_w, fc2_b = f32(inputs['fc2_w']), f32(inputs['fc2_b'])
    ln1_s, ln1_b = f32(inputs['ln1_s']), f32(inputs['ln1_b'])
    ln2_s, ln2_b = f32(inputs['ln2_s']), f32(inputs['ln2_b'])

    wqk = np.empty((L, D, 2 * D), np.float16)
    wv = np.empty((L, D, D), np.float16)
    wp = np.empty((L, D, D), np.float16)
    w1 = np.empty((L, D, MLP), np.float16)
    w2 = np.empty((L, MLP, D), np.float16)
    bqk = np.empty((L, 2 * D), np.float32)
    b1 = np.empty((L, MLP), np.float32)
    bp = np.empty((L, D), np.float32)
    b2 = np.empty((L, D), np.float32)
    for l in range(L):
        swq = ln1_s[l][:, None] * qkv_w[l]
        bq_full = ln1_b[l] @ qkv_w[l] + qkv_b[l]
        wqk[l] = swq[:, :2 * D].astype(np.float16)
        wv[l] = swq[:, 2 * D:].astype(np.float16)
        bqk[l] = bq_full[:2 * D]
        bv = bq_full[2 * D:]
        wp[l] = proj_w[l].astype(np.float16)
        bp[l] = bv @ proj_w[l] + proj_b[l]
        w1[l] = (ln2_s[l][:, None] * fc1_w[l]).astype(np.float16)
        b1[l] = ln2_b[l] @ fc1_w[l] + fc1_b[l]
        w2[l] = fc2_w[l].astype(np.float16)
        b2[l] = fc2_b[l]
    norm_s, norm_b = f32(inputs['norm_s']), f32(inputs['norm_b'])
    head_w, head_b = f32(inputs['head_w']), f32(inputs['head_b'])
    wh = (norm_s[:, None] * head_w).astype(np.float16)
    bh = (norm_b @ head_w + head_b).astype(np.float32)
    pospb = (f32(inputs['pos_embed'])[0, 1:] + f32(inputs['patch_b'])[None, :]).astype(np.float32)
    clsrow = (f32(inputs['cls_token'])[0, 0] + f32(inputs['pos_embed'])[0, 0]).astype(np.float32)[None, :]
    wpatch = f32(inputs['patch_w']).astype(np.float16)
    has_bias2 = bool(np.any(bp) or np.any(b2) or np.any(bh))
    return dict(wqk=wqk, wv=wv, wp=wp, w1=w1, w2=w2, bqk=bqk, b1=b1, bp=bp, b2=b2,
                wh=wh, bh=bh, pospb=pospb, clsrow=clsrow, wpatch=wpatch,
                has_bias2=has_bias2)


def _rearrange_kp(a, p=128):
    """[K, N] -> [p, K//p, N] partition-major layout for SBUF staging."""
    K, N = a.shape
    assert K % p == 0
    return np.ascontiguousarray(a.reshape(K // p, p, N).transpose(1, 0, 2))


def _host_inputs_per_core(inputs, prep, sched_T, keeps, img):
    x = np.asarray(inputs['x'], np.float32)[img]  # [C, IMG, IMG]
    patches = x.reshape(C, G, P, G, P).transpose(1, 3, 0, 2, 4).reshape(G * G, C * P * P)
    Tp0 = _pad128(G * G + 1)
    # column t = patch t-1; col 0 (CLS slot) and pad cols are zero, so the
    # patch GEMM directly produces aligned token tiles.
    patchesT_aug = np.zeros((C * P * P, Tp0), np.float16)
    patchesT_aug[:, 1:G * G + 1] = patches.T.astype(np.float16)
    pospb_aug = np.zeros((Tp0, D), np.float32)
    pospb_aug[0] = prep['clsrow'][0]
    pospb_aug[1:G * G + 1] = prep['pospb']
    m = {
        'patchesT': np.ascontiguousarray(
            patchesT_aug.reshape(6, 128, Tp0).transpose(1, 0, 2)),  # [128, 6, Tp0]
        'wpatch': _rearrange_kp(prep['wpatch']),                    # [128, 6, 384]
        'pospb': pospb_aug,
        'wqk': np.stack([_rearrange_kp(prep['wqk'][l]) for l in range(L)]),
        'wv': np.stack([_rearrange_kp(prep['wv'][l]) for l in range(L)]),
        'wp': np.stack([_rearrange_kp(prep['wp'][l]) for l in range(L)]),
        'w1': np.stack([_rearrange_kp(prep['w1'][l]) for l in range(L)]),
        'w2': np.stack([_rearrange_kp(prep['w2'][l]) for l in range(L)]),
        'bqk': np.stack([np.ascontiguousarray(prep['bqk'][l].reshape(6, 128).T) for l in range(L)]),
        'b1': np.stack([np.ascontiguousarray(prep['b1'][l].reshape(12, 128).T) for l in range(L)]),
        'wh': _rearrange_kp(prep['wh']),
    }
    for l in range(L):
        if keeps[l] is not None:
            Tn = len(keeps[l])
            idx = np.zeros((_pad128(Tn),), np.int32)
            idx[:Tn] = keeps[l]
            m[f'kidx{l}'] = np.ascontiguousarray(idx.reshape(-1, 128).T)  # [128, nMn]
    return m


# --------------------------------------------------------------------------
# Graph builder
# --------------------------------------------------------------------------
def build_graph(sched_T, keeps, nlayers=L, debug_taps=False):
    _install_patches()
    nc = bacc.Bacc("TRN2", target_bir_lowering=False, debug=False, num_devices=B)

    ext = {}
    Tp0 = _pad128(G * G + 1)
    ext['patchesT'] = nc.dram_tensor('patchesT', [128, 6, Tp0], F16, kind="ExternalInput")
    ext['wpatch'] = nc.dram_tensor('wpatch', [128, 6, D], F16, kind="ExternalInput")
    ext['pospb'] = nc.dram_tensor('pospb', [Tp0, D], F32, kind="ExternalInput")
    ext['wqk'] = nc.dram_tensor('wqk', [L, 128, 3, 2 * D], F16, kind="ExternalInput")
    ext['wv'] = nc.dram_tensor('wv', [L, 128, 3, D], F16, kind="ExternalInput")
    ext['wp'] = nc.dram_tensor('wp', [L, 128, 3, D], F16, kind="ExternalInput")
    ext['w1'] = nc.dram_tensor('w1', [L, 128, 3, MLP], F16, kind="ExternalInput")
    ext['w2'] = nc.dram_tensor('w2', [L, 128, 12, D], F16, kind="ExternalInput")
    ext['bqk'] = nc.dram_tensor('bqk', [L, 128, 6], F32, kind="ExternalInput")
    ext['b1'] = nc.dram_tensor('b1', [L, 128, 12], F32, kind="ExternalInput")
    ext['wh'] = nc.dram_tensor('wh', [128, 3, NCLS], F16, kind="ExternalInput")
    for l in range(nlayers):
        if keeps[l] is not None and l + 1 < nlayers:
            nMn = _pad128(len(keeps[l])) // 128
            ext[f'kidx{l}'] = nc.dram_tensor(f'kidx{l}', [128, nMn], I32, kind="ExternalInput")
    out_ext = nc.dram_tensor('out', [1, NCLS], F32, kind="ExternalOutput")
    taps = []
    if debug_taps:
        for l in range(nlayers):
            Tl = sched_T[l]
            taps.append(nc.dram_tensor(f'tap{l}', [Tl, D], F32, kind="ExternalOutput"))
        taps_mid = [nc.dram_tensor(f'tapmid{l}', [sched_T[l], D], F32, kind="ExternalOutput")
                    for l in range(nlayers)]
        tap_emb = nc.dram_tensor('tapemb', [sched_T[0], D], F32, kind="ExternalOutput")
        taps = (taps, taps_mid, tap_emb)

    xdram = {}
    for l in range(nlayers):
        if keeps[l] is not None and l + 1 < nlayers:
            xdram[l] = nc.dram_tensor(f'xspill{l}', [sched_T[l], D], F32)

    with tile.TileContext(nc) as tc:
        _build_body(nc, tc, ext, out_ext, xdram, sched_T, keeps, nlayers, taps)

    nc.compile()
    return nc


def _build_body(nc, tc, ext, out_ext, xdram, sched_T, keeps, nlayers, taps):
    import contextlib
    taps_mid = tap_emb = None
    if taps:
        taps, taps_mid, tap_emb = taps
    stack = contextlib.ExitStack()
    with stack:
        const = stack.enter_context(tc.tile_pool(name="const", bufs=1))
        wpool = stack.enter_context(tc.tile_pool(name="w", bufs=2))
        xpool = stack.enter_context(tc.tile_pool(name="x", bufs=12))
        apool = stack.enter_context(tc.tile_pool(name="act", bufs=3))
        vpool = stack.enter_context(tc.tile_pool(name="v", bufs=6))
        qpool = stack.enter_context(tc.tile_pool(name="q", bufs=7))
        hpool = stack.enter_context(tc.tile_pool(name="h", bufs=13))
        ppool = stack.enter_context(tc.tile_pool(name="probs", bufs=8))
        spool = stack.enter_context(tc.tile_pool(name="small", bufs=6))
        psA = stack.enter_context(tc.tile_pool(name="psA", bufs=2, space="PSUM"))
        psB = stack.enter_context(tc.tile_pool(name="psB", bufs=2, space="PSUM"))
        psC = stack.enter_context(tc.tile_pool(name="psC", bufs=1, space="PSUM"))

        ident = const.tile([128, 128], F16)
        make_identity(nc, ident[:])
        eps_c = const.tile([128, 1], F32, name="eps_c")
        nc.vector.memset(eps_c[:], float(LN_EPS))
        ones16 = const.tile([128, 64], F16, name="ones16")
        nc.vector.memset(ones16[:], 1.0)

        # ---------------- patch embed ----------------
        T = sched_T[0]
        Tp = _pad128(T)
        nM = Tp // 128
        pt = const.tile([128, 6, Tp], F16, tag="patchesT")
        nc.sync.dma_start(out=pt[:], in_=ext['patchesT'][:])
        wpt = const.tile([128, 6, D], F16, tag="wpatch", name="wpt")
        nc.sync.dma_start(out=wpt[:], in_=ext['wpatch'][:])

        xcur = [xpool.tile([128, D], F32, tag="xcur", name=f"xcur_pe_{mt}") for mt in range(nM)]
        pospb_sb = const.tile([128, nM, D], F32, tag="pospb", name="pospb_sb")
        nc.sync.dma_start(out=pospb_sb[:],
                          in_=ext['pospb'][:].rearrange("(m p) d -> p m d", p=128))
        for mt in range(nM):
            ps = psB.tile([128, D], F32, tag="sml")
            for k in range(6):
                nc.tensor.matmul(
                    out=ps[:],
                    lhsT=pt[:, k, mt * 128:(mt + 1) * 128],
                    rhs=wpt[:, k, :],
                    start=(k == 0), stop=(k == 5),
                )
            nc.vector.tensor_add(
                out=xcur[mt][:], in0=ps[:], in1=pospb_sb[:, mt, :],
            )
        if tap_emb is not None:
            for mt in range(nM):
                rows = min(128, T - mt * 128)
                nc.sync.dma_start(out=tap_emb[mt * 128:mt * 128 + rows, :],
                                  in_=xcur[mt][:rows, :])

        # ---------------- transformer layers ----------------
        for l in range(nlayers):
            T = sched_T[l]
            Tp = _pad128(T)
            nM = Tp // 128
            cls_only = (l == L - 1) and (nlayers == L)

            wqk_sb = wpool.tile([128, 3, 2 * D], F16, tag="wqk")
            nc.sync.dma_start(out=wqk_sb[:], in_=ext['wqk'][l])
            wv_sb = wpool.tile([128, 3, D], F16, tag="wv")
            nc.sync.dma_start(out=wv_sb[:], in_=ext['wv'][l])
            wp_sb = wpool.tile([128, 3, D], F16, tag="wp")
            nc.sync.dma_start(out=wp_sb[:], in_=ext['wp'][l])
            w1_sb = wpool.tile([128, 3, MLP], F16, tag="w1")
            nc.sync.dma_start(out=w1_sb[:], in_=ext['w1'][l])
            w2_sb = wpool.tile([128, 12, D], F16, tag="w2")
            nc.sync.dma_start(out=w2_sb[:], in_=ext['w2'][l])
            bqk_sb = wpool.tile([128, 6], F32, tag="bqk")
            nc.sync.dma_start(out=bqk_sb[:], in_=ext['bqk'][l])
            b1_sb = wpool.tile([128, 12], F32, tag="b1")
            nc.sync.dma_start(out=b1_sb[:], in_=ext['b1'][l])

            # ---- LN1 -> x16 -> xT16 (feature-major) ----
            x16 = [vpool.tile([128, D], F16, tag="x16", name=f"x16_{l}_{mt}") for mt in range(nM)]
            for mt in range(nM):
                _ln_tiles(nc, spool, xcur[mt], x16[mt], eps_c)
            xT16 = _transpose_tiles(nc, apool, psB, ident, x16, nM, Tp, T, tag="xT16")

            # ---- QK GEMM -> qk16 feature-major [6][128, *] ----
            qk16 = []
            for m in range(6):
                qw = 1 if (cls_only and m < 3) else Tp
                ps = psA.tile([128, qw], F32, tag="big")
                for k in range(3):
                    for nch in range(0, qw, 512):
                        ne = min(nch + 512, qw)
                        nc.tensor.matmul(
                            out=ps[:, nch:ne],
                            lhsT=wqk_sb[:, k, m * 128:(m + 1) * 128],
                            rhs=xT16[k][:, nch:ne],
                            start=(k == 0), stop=(k == 2),
                        )
                q16 = qpool.tile([128, qw], F16, tag="qk16")
                nc.scalar.activation(out=q16[:], in_=ps[:], func=AF.Identity,
                                     bias=bqk_sb[:, m:m + 1], scale=1.0)
                qk16.append(q16)

            # ---- V GEMM -> v16 token-major [nM][128, 6, 65] (ones col @64) ----
            v16 = []
            for mt in range(nM):
                rows = min(128, T - mt * 128)
                ps = psB.tile([128, D], F32, tag="sml")
                for k in range(3):
                    nc.tensor.matmul(
                        out=ps[:rows, :], lhsT=xT16[k][:, mt * 128:mt * 128 + rows],
                        rhs=wv_sb[:, k, :], start=(k == 0), stop=(k == 2),
                    )
                vt = vpool.tile([128, 6, 65], F16, tag="v16")
                if rows < 128:
                    nc.vector.memset(vt[:], 0.0)
                nc.vector.memset(vt[:rows, :, 64:65], 1.0)
                nc.vector.tensor_copy(
                    out=vt[:rows, :, 0:64],
                    in_=ps[:rows, :].rearrange("p (h d) -> p h d", h=6),
                )
                v16.append(vt)

            # ---- attention per head ----
            nQ = 1 if cls_only else Tp
            o16 = [apool.tile([128, nQ], F16, tag="o16", name=f"o16_{l}_{k}") for k in range(3)]
            for h in range(6):
                pprob = []
                for mt in range(nM):
                    rows = min(128, T - mt * 128)
                    ps = psA.tile([128, nQ], F32, tag="big")
                    for nch in range(0, nQ, 512):
                        ne = min(nch + 512, nQ)
                        nc.tensor.matmul(
                            out=ps[:rows, nch:ne],
                            lhsT=qk16[3 + h // 2][(h % 2) * 64:(h % 2) * 64 + 64,
                                                  mt * 128:mt * 128 + rows],
                            rhs=qk16[h // 2][(h % 2) * 64:(h % 2) * 64 + 64, nch:ne],
                            start=True, stop=True,
                        )
                    pb = ppool.tile([128, nQ], F16, tag="probs")
                    nc.scalar.activation(out=pb[:rows, :], in_=ps[:rows, :],
                                         func=AF.Exp, scale=float(SCALE))
                    pprob.append(pb)
                psav = psA.tile([65, nQ], F32, tag="big")
                for mt in range(nM):
                    rows = min(128, T - mt * 128)
                    for nch in range(0, nQ, 512):
                        ne = min(nch + 512, nQ)
                        nc.tensor.matmul(
                            out=psav[:, nch:ne],
                            lhsT=v16[mt][:rows, h, :],
                            rhs=pprob[mt][:rows, nch:ne],
                            start=(mt == 0), stop=(mt == nM - 1),
                        )
                rrow = spool.tile([1, nQ], F16, tag="rrow")
                with nc.allow_low_precision(reason="softmax 1/sum at fp16 matches the fp16 noise floor"):
                    nc.vector.reciprocal(out=rrow[0:1, :], in_=psav[64:65, :])
                rrep = psC.tile([64, nQ], F32, tag="rrep")
                for nch in range(0, nQ, 512):
                    ne = min(nch + 512, nQ)
                    nc.tensor.matmul(out=rrep[:, nch:ne], lhsT=ones16[0:1, :],
                                     rhs=rrow[0:1, nch:ne], start=True, stop=True)
                o16u = spool.tile([64, nQ], F16, tag="o16u")
                nc.scalar.copy(out=o16u[:], in_=psav[0:64, :])
                nc.vector.tensor_tensor(
                    out=o16[h // 2][(h % 2) * 64:(h % 2) * 64 + 64, :],
                    in0=o16u[:], in1=rrep[:], op=mybir.AluOpType.mult,
                )

            # ---- proj + residual ----
            nMq = 1 if cls_only else nM
            for mt in range(nMq):
                rows = 1 if cls_only else min(128, T - mt * 128)
                ps = psB.tile([128, D], F32, tag="sml")
                for k in range(3):
                    nc.tensor.matmul(
                        out=ps[:rows, :], lhsT=o16[k][:, mt * 128:mt * 128 + rows],
                        rhs=wp_sb[:, k, :], start=(k == 0), stop=(k == 2),
                    )
                nc.vector.tensor_add(out=xcur[mt][:rows, :], in0=xcur[mt][:rows, :],
                                     in1=ps[:rows, :])
            if taps_mid is not None:
                for mt in range(nMq):
                    rows = 1 if cls_only else min(128, T - mt * 128)
                    nc.sync.dma_start(out=taps_mid[l][mt * 128:mt * 128 + rows, :],
                                      in_=xcur[mt][:rows, :])

            # ---- LN2 -> MLP ----
            x216 = [vpool.tile([128, D], F16, tag="x16", name=f"x216_{l}_{mt}") for mt in range(nMq)]
            for mt in range(nMq):
                _ln_tiles(nc, spool, xcur[mt], x216[mt], eps_c, rows=(1 if cls_only else None))
            if cls_only:
                x2T = _transpose_cls(nc, apool, psB, ident, x216[0])
                nQm = 1
            else:
                x2T = _transpose_tiles(nc, apool, psB, ident, x216, nM, Tp, T, tag="x2T16")
                nQm = Tp
            h16 = []
            for m in range(12):
                ps = psA.tile([128, nQm], F32, tag="big")
                for k in range(3):
                    for nch in range(0, nQm, 512):
                        ne = min(nch + 512, nQm)
                        nc.tensor.matmul(
                            out=ps[:, nch:ne], lhsT=w1_sb[:, k, m * 128:(m + 1) * 128],
                            rhs=x2T[k][:, nch:ne], start=(k == 0), stop=(k == 2),
                        )
                ht = hpool.tile([128, nQm], F16, tag="h16")
                nc.scalar.activation(out=ht[:], in_=ps[:], func=AF.Gelu,
                                     bias=b1_sb[:, m:m + 1], scale=1.0)
                h16.append(ht)
            for mt in range(nMq):
                rows = 1 if cls_only else min(128, T - mt * 128)
                ps = psB.tile([128, D], F32, tag="sml")
                for k in range(12):
                    nc.tensor.matmul(
                        out=ps[:rows, :], lhsT=h16[k][:, mt * 128:mt * 128 + rows],
                        rhs=w2_sb[:, k, :], start=(k == 0), stop=(k == 11),
                    )
                nc.vector.tensor_add(out=xcur[mt][:rows, :], in0=xcur[mt][:rows, :],
                                     in1=ps[:rows, :])

            if taps:
                for mt in range(nM if not cls_only else 1):
                    rows = min(128, T - mt * 128) if not cls_only else 1
                    nc.sync.dma_start(out=taps[l][mt * 128:mt * 128 + rows, :],
                                      in_=xcur[mt][:rows, :])

            # ---- pruning gather ----
            if keeps[l] is not None and l + 1 < nlayers:
                Tn = sched_T[l + 1]
                nMn = _pad128(Tn) // 128
                for mt in range(nM):
                    rows = min(128, T - mt * 128)
                    nc.sync.dma_start(out=xdram[l][mt * 128:mt * 128 + rows, :],
                                      in_=xcur[mt][:rows, :])
                idx_sb = spool.tile([128, nMn], I32, tag="kidx")
                nc.sync.dma_start(out=idx_sb[:], in_=ext[f'kidx{l}'][:])
                xnew = [xpool.tile([128, D], F32, tag="xcur", name=f"xcur_{l}_{mt}") for mt in range(nMn)]
                for mt in range(nMn):
                    rows = min(128, Tn - mt * 128)
                    if rows < 128:
                        nc.vector.memset(xnew[mt][:], 0.0)
                    nc.gpsimd.indirect_dma_start(
                        out=xnew[mt][:rows, :],
                        out_offset=None,
                        in_=xdram[l][:, :],
                        in_offset=bass.IndirectOffsetOnAxis(ap=idx_sb[:rows, mt:mt + 1], axis=0),
                    )
                xcur = xnew

        # ---------------- final LN + head ----------------
        wh_sb = const.tile([128, 3, NCLS], F16, tag="wh", name="wh_sb")
        nc.sync.dma_start(out=wh_sb[:], in_=ext['wh'][:])
        xf16 = vpool.tile([128, D], F16, tag="x16")
        _ln_tiles(nc, spool, xcur[0], xf16, eps_c, rows=1)
        xfT = _transpose_cls(nc, apool, psB, ident, xf16)
        osb = const.tile([1, NCLS], F32, tag="osb", name="osb")
        for nch in range(0, NCLS, 500):
            ne = min(nch + 500, NCLS)
            pso = psB.tile([1, 500], F32, tag="sml")
            for k in range(3):
                nc.tensor.matmul(out=pso[:, :ne - nch], lhsT=xfT[k][:, 0:1],
                                 rhs=wh_sb[:, k, nch:ne], start=(k == 0), stop=(k == 2))
            nc.scalar.copy(out=osb[:, nch:ne], in_=pso[:, :ne - nch])
        nc.sync.dma_start(out=out_ext[:], in_=osb[:])


def _ln_tiles(nc, spool, xin, x16out, eps_c=None, rows=None):
    """LayerNorm stats on fp32 token-major tile -> fp16 normalized output
    (LN scale/bias are folded into downstream weights on the host)."""
    r = 128 if rows is None else rows
    st6 = spool.tile([128, 6], F32, tag="st6")
    st2 = spool.tile([128, 2], F32, tag="st2")
    nc.vector.bn_stats(out=st6[:r, :], in_=xin[:r, :])
    nc.vector.bn_aggr(out=st2[:r, :], in_=st6[:r, :])
    sd = spool.tile([128, 1], F32, tag="sd")
    nc.scalar.activation(out=sd[:r, :], in_=st2[:r, 1:2], func=AF.Sqrt, bias=eps_c[:r, :])
    rstd = spool.tile([128, 1], F32, tag="rstd")
    nc.vector.reciprocal(out=rstd[:r, :], in_=sd[:r, :])
    negmr = spool.tile([128, 1], F32, tag="negmr")
    nc.vector.tensor_tensor(out=negmr[:r, :], in0=st2[:r, 0:1], in1=rstd[:r, :],
                            op=mybir.AluOpType.mult)
    nc.vector.tensor_scalar_mul(negmr[:r, :], negmr[:r, :], -1.0)
    nc.scalar.activation(out=x16out[:r, :], in_=xin[:r, :], func=AF.Identity,
                         bias=negmr[:r, :], scale=rstd[:r, :])


def _transpose_tiles(nc, apool, psB, ident, x16, nM, Tp, T, tag):
    """token-major fp16 [nM][128, 384] -> feature-major [3][128, Tp];
    zero the pad columns (tokens beyond T)."""
    xT = [apool.tile([128, Tp], F16, tag=tag, name=f"{tag}_{id(x16)%100000}_{k}") for k in range(3)]
    for k in range(3):
        for mt in range(nM):
            pst = psB.tile([128, 128], F16, tag="sml")
            nc.tensor.transpose(out=pst[:], in_=x16[mt][:, k * 128:(k + 1) * 128],
                                identity=ident[:])
            nc.vector.tensor_copy(out=xT[k][:, mt * 128:(mt + 1) * 128], in_=pst[:])
        if Tp > T:
            nc.vector.memset(xT[k][:, T:], 0.0)
    return xT


def _transpose_cls(nc, apool, psB, ident, x16_tile):
    """[1, 384] fp16 row -> [3][128, 1] feature-major columns."""
    xT = [apool.tile([128, 1], F16, tag="clsT", name=f"clsT_{id(x16_tile)%100000}_{k}") for k in range(3)]
    for k in range(3):
        pst = psB.tile([128, 128], F16, tag="sml")
        nc.tensor.transpose(out=pst[:, 0:1], in_=x16_tile[0:1, k * 128:(k + 1) * 128],
                            identity=ident[0:1, 0:1])
        nc.vector.tensor_copy(out=xT[k][:], in_=pst[:, 0:1])
    return xT


# --------------------------------------------------------------------------
# NTFF profile hook (this container lacks antenv.axon_hooks)
# --------------------------------------------------------------------------
def install_ntff_hook():
    try:
        from trn_agent_boot.trn_boot import _ntff_profile_via_ctypes
        hook = _ntff_profile_via_ctypes('/opt/axon/libaxon_pjrt.so')
    except Exception:
        hook = None
    mod = types.ModuleType('antenv.axon_hooks')
    mod.get_axon_ntff_profile_hook = lambda: hook
    sys.modules['antenv.axon_hooks'] = mod


def _input_names(nc):
    names = set()
    for alloc in nc.m.functions[0].allocations:
        if isinstance(alloc, mybir.MemoryLocationSet) and alloc.kind == "ExternalInput":
            names.add(alloc.memorylocations[0].name)
    return names


# --------------------------------------------------------------------------
# Entry point
# --------------------------------------------------------------------------
def kernel(nlayers=L, trace=False, debug_taps=False, _return_res=False, **inputs):
    sched_T, keeps = _host_schedule(inputs)
    prep = _prep_weights(inputs)
    if prep['has_bias2']:
        raise NotImplementedError(
            "proj/fc2/head biases are all zero in this model family; "
            "nonzero values would need the ones-row bias path")
    nc = build_graph(sched_T, keeps, nlayers=nlayers, debug_taps=debug_taps)
    names = _input_names(nc)
    in_maps = []
    for img in range(B):
        m = _host_inputs_per_core(inputs, prep, sched_T, keeps, img)
        in_maps.append({k: v for k, v in m.items() if k in names})
    if trace:
        install_ntff_hook()
    res = run_bass_kernel_spmd(nc, in_maps, core_ids=list(range(B)), trace=trace)
    out = np.stack([res.results[i]['out'][0] for i in range(B)])
    if _return_res:
        return out, res
    return out
